# revision 23
# baseline (speedup 1.0000x reference)
"""2-layer weighted-GCN embedding kernel for 8 Trainium2 NeuronCores.

Strategy (dst-sharded message passing):
  - Nodes are sharded by destination across the 8 cores (12500 each, padded
    to 12544 = 98 * 128).  Each core handles every edge whose dst lands in
    its shard, so the scatter-add is purely local.
  - GCN associativity: conv(x) = (A_hat @ x) @ W^T + b, so we aggregate RAW
    features first and apply the dense transform on the (sharded) aggregate.
  - Self-loops are appended as ordinary edges and the full symmetric
    normalization dinv[src]*w*dinv[dst] is folded into the per-edge
    indicator value, so no normalization happens on device at all.
  - Per-edge gather of source rows uses the SWDGE dma_gather instruction
    (bf16 rows, 256 B each).  Indices are int16, so the padded node table
    (100352 rows) is split into 4 chunks of 25088 rows.  The 4 per-supertile
    chunk gathers run on SWDGE queues 0-3 so descriptor generation uses all
    four Q7 core pairs concurrently.
  - Scatter-add is an indicator matmul with the aggregate kept transposed:
    for each block of 128 edge slots, DVE builds ind[e, j] =
    (j == dst_rel[e]) * norm[e] on-chip (iota compare), and the tensor
    engine accumulates msg^T @ ind = agg_T[f, dst] into PSUM.  agg_T feeds
    the dense W matmul directly (contraction over f on partitions), so no
    transpose is needed inside a layer; only the layer-1 hidden state is
    transposed once (identity matmul) for its row-major HBM store.
  - Between the two conv layers one AllGather shares r1 = relu(h1) across
    cores (bf16).
  - The final output is produced transposed ([ENC, shard]) and transposed
    back on the host.

kernel(**inputs) takes the FULL inputs and returns the FULL [100000, 64]
output; everything (sharding, compile, SPMD run, gather of shards) happens
inside.
"""

import numpy as np
import ml_dtypes

import concourse.bass as bass
import concourse.tile as tile
import concourse.bacc as bacc
from concourse import mybir, bass_utils

BF16 = ml_dtypes.bfloat16

F = 128
HID = 128
ENC = 64
NCORES = 8
SUBW = 128
SUPSZ = 6                      # subtiles per supertile (one gather covers these)


def _set_dims(n):
    """(Re)compute the node-count-derived global dims. Called at import with
    the real N; tests may call with a tiny N."""
    global N, SHARD, NSUB, SHARD_PAD, CHUNK, XROWS, NSUP
    N = n
    SHARD = N // NCORES
    NSUB = -(-SHARD // SUBW)           # subtiles per shard
    SHARD_PAD = NSUB * SUBW
    CHUNK = 2 * SHARD_PAD              # rows per gather chunk (< 2**15)
    XROWS = NCORES * SHARD_PAD         # padded node-table rows
    NSUP = -(-NSUB // SUPSZ)


NCHUNK = 4
_set_dims(100000)

_cache = {}


def _preprocess(x, edge_index, edge_weight, W1, b1, W2, b2, Wf, bf):
    """All host-side numpy prep: normalization, edge partitioning, layouts."""
    src = np.asarray(edge_index[0], dtype=np.int64)
    dst = np.asarray(edge_index[1], dtype=np.int64)
    w = np.asarray(edge_weight, dtype=np.float32)
    x = np.asarray(x, dtype=np.float32)

    deg = np.bincount(dst, weights=w.astype(np.float64), minlength=N) + 1.0
    dinv = (1.0 / np.sqrt(deg)).astype(np.float32)

    # self-loops as ordinary edges; full norm folded into the edge value
    loop = np.arange(N, dtype=np.int64)
    src_f = np.concatenate([src, loop])
    dst_f = np.concatenate([dst, loop])
    norm_f = np.concatenate([dinv[src] * w * dinv[dst], dinv * dinv])

    x_pad = np.zeros((XROWS, F), np.float32)
    for o in range(NCORES):
        x_pad[o * SHARD_PAD:o * SHARD_PAD + SHARD] = x[o * SHARD:(o + 1) * SHARD]
    x_bf = x_pad.astype(BF16)

    # map src node id -> (chunk, local row) in the padded table
    owner = src_f // SHARD
    src_pad = owner * SHARD_PAD + (src_f - owner * SHARD)
    chunk = src_pad // CHUNK
    src_local = (src_pad - chunk * CHUNK).astype(np.int64)
    assert src_local.max() < 2 ** 15

    NCELL = NCHUNK * NSUB  # flat cell id = c * NSUB + t

    # per-device cell contents
    dev = []
    counts = np.zeros((NCORES, NCELL), np.int64)
    for d in range(NCORES):
        lo, hi = d * SHARD, (d + 1) * SHARD
        m = (dst_f >= lo) & (dst_f < hi)
        dl = dst_f[m] - lo
        t = dl // SUBW
        cid = chunk[m] * NSUB + t
        order = np.argsort(cid, kind="stable")
        cid_s = cid[order]
        counts[d] = np.bincount(cid_s, minlength=NCELL)
        dev.append((cid_s,
                    src_local[m][order].astype(np.int16),
                    (dl % SUBW)[order].astype(np.float32),
                    norm_f[m][order]))

    nb_cell = -(-counts.max(axis=0) // 128)            # blocks per cell (shared)
    cell_off = np.zeros(NCELL + 1, np.int64)
    np.cumsum(nb_cell * 128, out=cell_off[1:])
    TOT = int(cell_off[-1])

    per_core = []
    for d in range(NCORES):
        cid_s, sl, dr, nm = dev[d]
        starts = np.zeros(NCELL + 1, np.int64)
        np.cumsum(counts[d], out=starts[1:])
        rank = np.arange(len(cid_s)) - starts[cid_s]
        pos = cell_off[cid_s] + rank
        f_src = np.zeros(TOT, np.int16)
        f_dr = np.zeros(TOT, np.float32)
        f_nm = np.zeros(TOT, np.float32)
        f_src[pos] = sl
        f_dr[pos] = dr
        f_nm[pos] = nm

        idx16 = np.ascontiguousarray(np.tile(f_src.reshape(-1, 16).T, (8, 1)))
        # host-built indicators, partition-major: indb[p, blk*128 + dst_rel]
        # = norm (slot = blk*128 + p; one matmul block = columns
        # [blk*128, (blk+1)*128))
        indb = np.zeros((128, TOT), BF16)
        pos = np.arange(TOT)
        indb[pos % 128, (pos // 128) * 128 + f_dr.astype(np.int64)] = \
            f_nm.astype(BF16)

        per_core.append({
            "idx16": idx16,
            "indb": indb,
        })

    shared = {
        "x_bf": x_bf,
        "w1t": np.ascontiguousarray(np.asarray(W1, np.float32).T.astype(BF16)),
        "w2t": np.ascontiguousarray(np.asarray(W2, np.float32).T.astype(BF16)),
        "wft": np.ascontiguousarray(np.asarray(Wf, np.float32).T.astype(BF16)),
        "b1c": np.asarray(b1, np.float32).reshape(HID, 1).copy(),
        "b2c": np.asarray(b2, np.float32).reshape(HID, 1).copy(),
        "bfc": np.asarray(bf, np.float32).reshape(ENC, 1).copy(),
        "identb": np.eye(128, dtype=np.float32).astype(BF16),
    }
    nb = nb_cell.reshape(NCHUNK, NSUB)      # [c][t]
    offs = cell_off.reshape(-1)             # flat slot offsets, id = c*NSUB+t
    return shared, per_core, nb, offs, TOT


def _build(nb, offs, TOT):
    """Build the SPMD bass program (identical for all 8 cores)."""
    nc = bacc.Bacc("TRN2", target_bir_lowering=False, debug=False,
                   num_devices=NCORES, num_swdge_queues=4,
                   dynamic_dma_scratch_size=32768)
    f32 = mybir.dt.float32
    bf16 = mybir.dt.bfloat16

    x_bf_t = nc.dram_tensor("x_bf", [XROWS, F], bf16, kind="ExternalInput")
    idx16_t = nc.dram_tensor("idx16", [128, TOT // 16], mybir.dt.int16, kind="ExternalInput")
    indb_t = nc.dram_tensor("indb", [128, TOT], bf16, kind="ExternalInput")
    w1t_t = nc.dram_tensor("w1t", [F, HID], bf16, kind="ExternalInput")
    w2t_t = nc.dram_tensor("w2t", [HID, HID], bf16, kind="ExternalInput")
    wft_t = nc.dram_tensor("wft", [HID, ENC], bf16, kind="ExternalInput")
    b1c_t = nc.dram_tensor("b1c", [HID, 1], f32, kind="ExternalInput")
    b2c_t = nc.dram_tensor("b2c", [HID, 1], f32, kind="ExternalInput")
    bfc_t = nc.dram_tensor("bfc", [ENC, 1], f32, kind="ExternalInput")
    identb_t = nc.dram_tensor("identb", [128, 128], bf16, kind="ExternalInput")
    out_t = nc.dram_tensor("out", [ENC, SHARD_PAD], f32, kind="ExternalOutput")

    # per-subtile block lists: blocks[t] = ordered [(c, k), ...]
    blocks = [[(c, k) for c in range(NCHUNK) for k in range(int(nb[c][t]))]
              for t in range(NSUB)]

    with tile.TileContext(nc) as tc:
        with tc.tile_pool(name="const", bufs=1) as cst, \
             tc.tile_pool(name="edata", bufs=1) as edata, \
             tc.tile_pool(name="msgp", bufs=3) as msgp, \
             tc.tile_pool(name="indp", bufs=6) as indp, \
             tc.tile_pool(name="accp", bufs=3, space="PSUM") as accp, \
             tc.tile_pool(name="epsp", bufs=2, space="PSUM") as epsp, \
             tc.tile_pool(name="work", bufs=3) as work, \
             tc.tile_pool(name="dram", bufs=1, space="DRAM") as dram:

            # ---- persistent SBUF data ----
            idx_sb = edata.tile([128, TOT // 16], mybir.dt.int16)
            nc.sync.dma_start(idx_sb[:], idx16_t[:])

            w1t_sb = cst.tile([F, HID], bf16)
            w2t_sb = cst.tile([HID, HID], bf16)
            wft_sb = cst.tile([HID, ENC], bf16)
            b1c_sb = cst.tile([HID, 1], f32)
            b2c_sb = cst.tile([HID, 1], f32)
            bfc_sb = cst.tile([ENC, 1], f32)
            ident_sb = cst.tile([128, 128], bf16)
            for sb_, t_ in ((w1t_sb, w1t_t), (w2t_sb, w2t_t), (wft_sb, wft_t),
                            (b1c_sb, b1c_t), (b2c_sb, b2c_t), (bfc_sb, bfc_t),
                            (ident_sb, identb_t)):
                nc.sync.dma_start(sb_[:], t_[:])

            r1sh = dram.tile([SHARD_PAD, HID], bf16)
            r1full = dram.tile([XROWS, HID], bf16, addr_space="Shared")

            def aggregate_layer(src_dram, layer):
                """Gather + indicator-matmul aggregation + per-subtile epilogue.

                Block order is subtile-major so each subtile's PSUM
                accumulation group opens and closes before the next one
                starts (accumulation groups are bank-granular)."""
                for s in range(NSUP):
                    subs = list(range(s * SUPSZ, min((s + 1) * SUPSZ, NSUB)))
                    msgs = {}
                    inds = {}
                    starts = {}
                    for c in range(NCHUNK):
                        start_slot = int(offs[c * NSUB + subs[0]])
                        end_slot = int(offs[c * NSUB + subs[-1] + 1])
                        L = end_slot - start_slot
                        if L == 0:
                            continue
                        starts[c] = start_slot
                        msg = msgp.tile([128, L], bf16, tag=f"msg{c}", bufs=2)
                        msgs[c] = msg
                        nc.gpsimd.dma_gather(
                            msg[:].rearrange("p (b f) -> p b f", f=128),
                            src_dram[c * CHUNK:(c + 1) * CHUNK, :],
                            idx_sb[:, start_slot // 16:end_slot // 16],
                            L, L, 128, elem_step=F,
                            single_packet=False,
                            queue_num=c,
                        )
                        ind = indp.tile([128, L], bf16, tag=f"ind{c}", bufs=2)
                        inds[c] = ind
                        nc.sync.dma_start(
                            ind[:], indb_t[:, start_slot:end_slot])

                    # per-supertile store staging keeps the sync-engine
                    # stream free of per-subtile stores (which would gate the
                    # next round's indicator loads behind this round's
                    # epilogue chain)
                    nsub_s = len(subs)
                    if layer == 0:
                        r1stage = work.tile([128, nsub_s * HID], bf16,
                                            tag="r1stage", bufs=2)
                    else:
                        ostage = work.tile([ENC, nsub_s * 128], f32,
                                           tag="ostage", bufs=2)

                    # ---- per-subtile accumulate + drain ----
                    for j, t in enumerate(subs):
                        acc = accp.tile([128, 128], f32, tag="acc")
                        for c, k in blocks[t]:
                            base = int(offs[c * NSUB + t])
                            mloc = (base - starts[c]) // 128 + k
                            nc.tensor.matmul(
                                acc[:],
                                lhsT=msgs[c][:, mloc * 128:(mloc + 1) * 128],
                                rhs=inds[c][:, mloc * 128:(mloc + 1) * 128],
                                start=(blocks[t][0] == (c, k)),
                                stop=(blocks[t][-1] == (c, k)),
                            )

                        # acc = agg_T [f, dst] in PSUM
                        aggT = work.tile([128, 128], bf16, tag="aggT")
                        nc.scalar.activation(aggT[:], acc[:],
                                             mybir.ActivationFunctionType.Copy)
                        wsb = w1t_sb if layer == 0 else w2t_sb
                        hp = epsp.tile([HID, 128], f32, tag="eps")
                        nc.tensor.matmul(hp[:], lhsT=wsb[:], rhs=aggT[:],
                                         start=True, stop=True)
                        bcol_sb = b1c_sb if layer == 0 else b2c_sb
                        rT = work.tile([HID, 128], bf16, tag="rT")
                        nc.scalar.activation(rT[:], hp[:],
                                             mybir.ActivationFunctionType.Relu,
                                             bias=bcol_sb[:])
                        if layer == 0:
                            # transpose to row-major for the HBM store
                            tp = epsp.tile([128, HID], f32, tag="eps")
                            nc.tensor.matmul(tp[:], lhsT=rT[:], rhs=ident_sb[:],
                                             start=True, stop=True)
                            nc.scalar.activation(
                                r1stage[:, j * HID:(j + 1) * HID], tp[:],
                                mybir.ActivationFunctionType.Copy)
                        else:
                            fp = epsp.tile([ENC, 128], f32, tag="epf")
                            nc.tensor.matmul(fp[:], lhsT=wft_sb[:], rhs=rT[:],
                                             start=True, stop=True)
                            nc.vector.tensor_scalar(
                                out=ostage[:, j * 128:(j + 1) * 128],
                                in0=fp[:],
                                scalar1=bfc_sb[:], scalar2=None,
                                op0=mybir.AluOpType.add)

                    t0 = subs[0]
                    if layer == 0:
                        nc.sync.dma_start(
                            r1sh[t0 * 128:(t0 + nsub_s) * 128, :].rearrange(
                                "(j p) f -> p j f", p=128),
                            r1stage[:].rearrange("p (j f) -> p j f", f=HID))
                    else:
                        nc.sync.dma_start(
                            out_t[:, t0 * 128:(t0 + nsub_s) * 128], ostage[:])

            aggregate_layer(x_bf_t, layer=0)
            nc.gpsimd.collective_compute(
                "AllGather",
                mybir.AluOpType.bypass,
                replica_groups=[list(range(NCORES))],
                ins=[r1sh[:].opt()],
                outs=[r1full[:].opt()],
            )
            aggregate_layer(r1full, layer=1)

    nc.compile()
    return nc


def kernel(**inputs):
    shared, per_core, nb, offs, TOT = _preprocess(
        inputs["x"], inputs["edge_index"], inputs["edge_weight"],
        inputs["W1"], inputs["b1"], inputs["W2"], inputs["b2"],
        inputs["Wf"], inputs["bf"])

    key = (TOT, nb.tobytes())
    if key not in _cache:
        _cache[key] = _build(nb, offs, TOT)
    nc = _cache[key]

    in_maps = []
    for d in range(NCORES):
        m = dict(shared)
        m.update(per_core[d])
        in_maps.append(m)

    res = bass_utils.run_bass_kernel_spmd(nc, in_maps, core_ids=list(range(NCORES)))
    out = np.concatenate(
        [res.results[d]["out"][:, :SHARD].T for d in range(NCORES)], axis=0)
    return np.ascontiguousarray(out, dtype=np.float32)


# revision 25
# speedup vs baseline: 1.0480x; 1.0480x over previous
"""2-layer weighted-GCN embedding kernel for 8 Trainium2 NeuronCores.

Strategy (dst-sharded message passing):
  - Nodes are sharded by destination across the 8 cores (12500 each, padded
    to 12544 = 98 * 128).  Each core handles every edge whose dst lands in
    its shard, so the scatter-add is purely local.
  - GCN associativity: conv(x) = (A_hat @ x) @ W^T + b, so we aggregate RAW
    features first and apply the dense transform on the (sharded) aggregate.
  - Self-loops are appended as ordinary edges and the full symmetric
    normalization dinv[src]*w*dinv[dst] is folded into the per-edge
    indicator value, so no normalization happens on device at all.
  - Per-edge gather of source rows uses the SWDGE dma_gather instruction
    (bf16 rows, 256 B each).  Indices are int16, so the padded node table
    (100352 rows) is split into 4 chunks of 25088 rows.  The 4 per-supertile
    chunk gathers run on SWDGE queues 0-3 so descriptor generation uses all
    four Q7 core pairs concurrently.
  - Scatter-add is an indicator matmul with the aggregate kept transposed:
    for each block of 128 edge slots, DVE builds ind[e, j] =
    (j == dst_rel[e]) * norm[e] on-chip (iota compare), and the tensor
    engine accumulates msg^T @ ind = agg_T[f, dst] into PSUM.  agg_T feeds
    the dense W matmul directly (contraction over f on partitions), so no
    transpose is needed inside a layer; only the layer-1 hidden state is
    transposed once (identity matmul) for its row-major HBM store.
  - Between the two conv layers one AllGather shares r1 = relu(h1) across
    cores (bf16).
  - The final output is produced transposed ([ENC, shard]) and transposed
    back on the host.

kernel(**inputs) takes the FULL inputs and returns the FULL [100000, 64]
output; everything (sharding, compile, SPMD run, gather of shards) happens
inside.
"""

import numpy as np
import ml_dtypes

import concourse.bass as bass
import concourse.tile as tile
import concourse.bacc as bacc
from concourse import mybir, bass_utils

BF16 = ml_dtypes.bfloat16

F = 128
HID = 128
ENC = 64
NCORES = 8
SUBW = 128
SUPSZ = 5                      # subtiles per supertile (one gather covers these)


def _set_dims(n):
    """(Re)compute the node-count-derived global dims. Called at import with
    the real N; tests may call with a tiny N."""
    global N, SHARD, NSUB, SHARD_PAD, CHUNK, XROWS, NSUP
    N = n
    SHARD = N // NCORES
    NSUB = -(-SHARD // SUBW)           # subtiles per shard
    SHARD_PAD = NSUB * SUBW
    CHUNK = 2 * SHARD_PAD              # rows per gather chunk (< 2**15)
    XROWS = NCORES * SHARD_PAD         # padded node-table rows
    NSUP = -(-NSUB // SUPSZ)


NCHUNK = 4
_set_dims(100000)

_cache = {}


def _preprocess(x, edge_index, edge_weight, W1, b1, W2, b2, Wf, bf):
    """All host-side numpy prep: normalization, edge partitioning, layouts."""
    src = np.asarray(edge_index[0], dtype=np.int64)
    dst = np.asarray(edge_index[1], dtype=np.int64)
    w = np.asarray(edge_weight, dtype=np.float32)
    x = np.asarray(x, dtype=np.float32)

    deg = np.bincount(dst, weights=w.astype(np.float64), minlength=N) + 1.0
    dinv = (1.0 / np.sqrt(deg)).astype(np.float32)

    # self-loops as ordinary edges; full norm folded into the edge value
    loop = np.arange(N, dtype=np.int64)
    src_f = np.concatenate([src, loop])
    dst_f = np.concatenate([dst, loop])
    norm_f = np.concatenate([dinv[src] * w * dinv[dst], dinv * dinv])

    x_pad = np.zeros((XROWS, F), np.float32)
    for o in range(NCORES):
        x_pad[o * SHARD_PAD:o * SHARD_PAD + SHARD] = x[o * SHARD:(o + 1) * SHARD]
    x_bf = x_pad.astype(BF16)

    # map src node id -> (chunk, local row) in the padded table
    owner = src_f // SHARD
    src_pad = owner * SHARD_PAD + (src_f - owner * SHARD)
    chunk = src_pad // CHUNK
    src_local = (src_pad - chunk * CHUNK).astype(np.int64)
    assert src_local.max() < 2 ** 15

    NCELL = NCHUNK * NSUB  # flat cell id = c * NSUB + t

    # per-device cell contents
    dev = []
    counts = np.zeros((NCORES, NCELL), np.int64)
    for d in range(NCORES):
        lo, hi = d * SHARD, (d + 1) * SHARD
        m = (dst_f >= lo) & (dst_f < hi)
        dl = dst_f[m] - lo
        t = dl // SUBW
        cid = chunk[m] * NSUB + t
        order = np.argsort(cid, kind="stable")
        cid_s = cid[order]
        counts[d] = np.bincount(cid_s, minlength=NCELL)
        dev.append((cid_s,
                    src_local[m][order].astype(np.int16),
                    (dl % SUBW)[order].astype(np.float32),
                    norm_f[m][order]))

    nb_cell = -(-counts.max(axis=0) // 128)            # blocks per cell (shared)
    cell_off = np.zeros(NCELL + 1, np.int64)
    np.cumsum(nb_cell * 128, out=cell_off[1:])
    TOT = int(cell_off[-1])

    per_core = []
    for d in range(NCORES):
        cid_s, sl, dr, nm = dev[d]
        starts = np.zeros(NCELL + 1, np.int64)
        np.cumsum(counts[d], out=starts[1:])
        rank = np.arange(len(cid_s)) - starts[cid_s]
        pos = cell_off[cid_s] + rank
        f_src = np.zeros(TOT, np.int16)
        f_dr = np.zeros(TOT, np.float32)
        f_nm = np.zeros(TOT, np.float32)
        f_src[pos] = sl
        f_dr[pos] = dr
        f_nm[pos] = nm

        idx16 = np.ascontiguousarray(np.tile(f_src.reshape(-1, 16).T, (8, 1)))
        # host-built indicators, partition-major: indb[p, blk*128 + dst_rel]
        # = norm (slot = blk*128 + p; one matmul block = columns
        # [blk*128, (blk+1)*128))
        indb = np.zeros((128, TOT), BF16)
        pos = np.arange(TOT)
        indb[pos % 128, (pos // 128) * 128 + f_dr.astype(np.int64)] = \
            f_nm.astype(BF16)

        per_core.append({
            "idx16": idx16,
            "indb": indb,
        })

    shared = {
        "x_bf": x_bf,
        "w1t": np.ascontiguousarray(np.asarray(W1, np.float32).T.astype(BF16)),
        "w2t": np.ascontiguousarray(np.asarray(W2, np.float32).T.astype(BF16)),
        "wft": np.ascontiguousarray(np.asarray(Wf, np.float32).T.astype(BF16)),
        "b1c": np.asarray(b1, np.float32).reshape(HID, 1).copy(),
        "b2c": np.asarray(b2, np.float32).reshape(HID, 1).copy(),
        "bfc": np.asarray(bf, np.float32).reshape(ENC, 1).copy(),
        "identb": np.eye(128, dtype=np.float32).astype(BF16),
    }
    nb = nb_cell.reshape(NCHUNK, NSUB)      # [c][t]
    offs = cell_off.reshape(-1)             # flat slot offsets, id = c*NSUB+t
    return shared, per_core, nb, offs, TOT


def _build(nb, offs, TOT):
    """Build the SPMD bass program (identical for all 8 cores)."""
    nc = bacc.Bacc("TRN2", target_bir_lowering=False, debug=False,
                   num_devices=NCORES, num_swdge_queues=4)
    f32 = mybir.dt.float32
    bf16 = mybir.dt.bfloat16

    x_bf_t = nc.dram_tensor("x_bf", [XROWS, F], bf16, kind="ExternalInput")
    idx16_t = nc.dram_tensor("idx16", [128, TOT // 16], mybir.dt.int16, kind="ExternalInput")
    indb_t = nc.dram_tensor("indb", [128, TOT], bf16, kind="ExternalInput")
    w1t_t = nc.dram_tensor("w1t", [F, HID], bf16, kind="ExternalInput")
    w2t_t = nc.dram_tensor("w2t", [HID, HID], bf16, kind="ExternalInput")
    wft_t = nc.dram_tensor("wft", [HID, ENC], bf16, kind="ExternalInput")
    b1c_t = nc.dram_tensor("b1c", [HID, 1], f32, kind="ExternalInput")
    b2c_t = nc.dram_tensor("b2c", [HID, 1], f32, kind="ExternalInput")
    bfc_t = nc.dram_tensor("bfc", [ENC, 1], f32, kind="ExternalInput")
    identb_t = nc.dram_tensor("identb", [128, 128], bf16, kind="ExternalInput")
    out_t = nc.dram_tensor("out", [ENC, SHARD_PAD], f32, kind="ExternalOutput")

    # per-subtile block lists: blocks[t] = ordered [(c, k), ...]
    blocks = [[(c, k) for c in range(NCHUNK) for k in range(int(nb[c][t]))]
              for t in range(NSUB)]

    with tile.TileContext(nc) as tc:
        with tc.tile_pool(name="const", bufs=1) as cst, \
             tc.tile_pool(name="edata", bufs=1) as edata, \
             tc.tile_pool(name="msgp", bufs=3) as msgp, \
             tc.tile_pool(name="indp", bufs=6) as indp, \
             tc.tile_pool(name="accp", bufs=3, space="PSUM") as accp, \
             tc.tile_pool(name="epsp", bufs=2, space="PSUM") as epsp, \
             tc.tile_pool(name="work", bufs=3) as work, \
             tc.tile_pool(name="dram", bufs=1, space="DRAM") as dram:

            # ---- persistent SBUF data ----
            idx_sb = edata.tile([128, TOT // 16], mybir.dt.int16)
            nc.sync.dma_start(idx_sb[:], idx16_t[:])

            w1t_sb = cst.tile([F, HID], bf16)
            w2t_sb = cst.tile([HID, HID], bf16)
            wft_sb = cst.tile([HID, ENC], bf16)
            b1c_sb = cst.tile([HID, 1], f32)
            b2c_sb = cst.tile([HID, 1], f32)
            bfc_sb = cst.tile([ENC, 1], f32)
            ident_sb = cst.tile([128, 128], bf16)
            for sb_, t_ in ((w1t_sb, w1t_t), (w2t_sb, w2t_t), (wft_sb, wft_t),
                            (b1c_sb, b1c_t), (b2c_sb, b2c_t), (bfc_sb, bfc_t),
                            (ident_sb, identb_t)):
                nc.sync.dma_start(sb_[:], t_[:])

            r1sh = dram.tile([SHARD_PAD, HID], bf16)
            r1full = dram.tile([XROWS, HID], bf16, addr_space="Shared")

            def aggregate_layer(src_dram, layer):
                """Gather + indicator-matmul aggregation + per-subtile epilogue.

                Block order is subtile-major so each subtile's PSUM
                accumulation group opens and closes before the next one
                starts (accumulation groups are bank-granular)."""
                for s in range(NSUP):
                    subs = list(range(s * SUPSZ, min((s + 1) * SUPSZ, NSUB)))
                    msgs = {}
                    inds = {}
                    starts = {}
                    for c in range(NCHUNK):
                        start_slot = int(offs[c * NSUB + subs[0]])
                        end_slot = int(offs[c * NSUB + subs[-1] + 1])
                        L = end_slot - start_slot
                        if L == 0:
                            continue
                        starts[c] = start_slot
                        msg = msgp.tile([128, L], bf16, tag=f"msg{c}", bufs=3)
                        msgs[c] = msg
                        nc.gpsimd.dma_gather(
                            msg[:].rearrange("p (b f) -> p b f", f=128),
                            src_dram[c * CHUNK:(c + 1) * CHUNK, :],
                            idx_sb[:, start_slot // 16:end_slot // 16],
                            L, L, 128, elem_step=F,
                            single_packet=False,
                            queue_num=c,
                        )
                        ind = indp.tile([128, L], bf16, tag=f"ind{c}", bufs=2)
                        inds[c] = ind
                        nc.sync.dma_start(
                            ind[:], indb_t[:, start_slot:end_slot])

                    # per-supertile store staging keeps the sync-engine
                    # stream free of per-subtile stores (which would gate the
                    # next round's indicator loads behind this round's
                    # epilogue chain)
                    nsub_s = len(subs)
                    if layer == 0:
                        r1stage = work.tile([128, nsub_s * HID], bf16,
                                            tag="r1stage", bufs=2)
                    else:
                        ostage = work.tile([ENC, nsub_s * 128], f32,
                                           tag="ostage", bufs=2)

                    # ---- per-subtile accumulate + drain ----
                    for j, t in enumerate(subs):
                        acc = accp.tile([128, 128], f32, tag="acc")
                        for c, k in blocks[t]:
                            base = int(offs[c * NSUB + t])
                            mloc = (base - starts[c]) // 128 + k
                            nc.tensor.matmul(
                                acc[:],
                                lhsT=msgs[c][:, mloc * 128:(mloc + 1) * 128],
                                rhs=inds[c][:, mloc * 128:(mloc + 1) * 128],
                                start=(blocks[t][0] == (c, k)),
                                stop=(blocks[t][-1] == (c, k)),
                            )

                        # acc = agg_T [f, dst] in PSUM
                        aggT = work.tile([128, 128], bf16, tag="aggT")
                        nc.scalar.activation(aggT[:], acc[:],
                                             mybir.ActivationFunctionType.Copy)
                        wsb = w1t_sb if layer == 0 else w2t_sb
                        hp = epsp.tile([HID, 128], f32, tag="eps")
                        nc.tensor.matmul(hp[:], lhsT=wsb[:], rhs=aggT[:],
                                         start=True, stop=True)
                        bcol_sb = b1c_sb if layer == 0 else b2c_sb
                        rT = work.tile([HID, 128], bf16, tag="rT")
                        nc.scalar.activation(rT[:], hp[:],
                                             mybir.ActivationFunctionType.Relu,
                                             bias=bcol_sb[:])
                        if layer == 0:
                            # transpose to row-major for the HBM store
                            tp = epsp.tile([128, HID], f32, tag="eps")
                            nc.tensor.matmul(tp[:], lhsT=rT[:], rhs=ident_sb[:],
                                             start=True, stop=True)
                            nc.scalar.activation(
                                r1stage[:, j * HID:(j + 1) * HID], tp[:],
                                mybir.ActivationFunctionType.Copy)
                        else:
                            fp = epsp.tile([ENC, 128], f32, tag="epf")
                            nc.tensor.matmul(fp[:], lhsT=wft_sb[:], rhs=rT[:],
                                             start=True, stop=True)
                            nc.vector.tensor_scalar(
                                out=ostage[:, j * 128:(j + 1) * 128],
                                in0=fp[:],
                                scalar1=bfc_sb[:], scalar2=None,
                                op0=mybir.AluOpType.add)

                    t0 = subs[0]
                    if layer == 0:
                        nc.sync.dma_start(
                            r1sh[t0 * 128:(t0 + nsub_s) * 128, :].rearrange(
                                "(j p) f -> p j f", p=128),
                            r1stage[:].rearrange("p (j f) -> p j f", f=HID))
                    else:
                        nc.sync.dma_start(
                            out_t[:, t0 * 128:(t0 + nsub_s) * 128], ostage[:])

            aggregate_layer(x_bf_t, layer=0)
            nc.gpsimd.collective_compute(
                "AllGather",
                mybir.AluOpType.bypass,
                replica_groups=[list(range(NCORES))],
                ins=[r1sh[:].opt()],
                outs=[r1full[:].opt()],
            )
            aggregate_layer(r1full, layer=1)

    nc.compile()
    return nc


def kernel(**inputs):
    shared, per_core, nb, offs, TOT = _preprocess(
        inputs["x"], inputs["edge_index"], inputs["edge_weight"],
        inputs["W1"], inputs["b1"], inputs["W2"], inputs["b2"],
        inputs["Wf"], inputs["bf"])

    key = (TOT, nb.tobytes())
    if key not in _cache:
        _cache[key] = _build(nb, offs, TOT)
    nc = _cache[key]

    in_maps = []
    for d in range(NCORES):
        m = dict(shared)
        m.update(per_core[d])
        in_maps.append(m)

    res = bass_utils.run_bass_kernel_spmd(nc, in_maps, core_ids=list(range(NCORES)))
    out = np.concatenate(
        [res.results[d]["out"][:, :SHARD].T for d in range(NCORES)], axis=0)
    return np.ascontiguousarray(out, dtype=np.float32)
